# revision 1
# baseline (speedup 1.0000x reference)
# Trainium2 Bass kernel for the 4-branch cross-attention block.
#
# Problem: N=4 batches, L1=L2=1024, D=512, H=8 heads of 64.
#   q1,k1,v1 = proj(input1); q2,k2,v2 = proj(input2)
#   four attention branches (q1k1v1, q1k2v2, q2k1v1, q2k2v2), masked softmax
#   over the key axis, outputs averaged pairwise.
#
# Sharding: 8 cores = 4 batches x 2 head-groups (4 heads each). SPMD — one
# program, per-core data.
#
# Device-side dataflow (per core, 16 branch-heads of L x L attention):
#   ST   = K @ Q^T          (keys on partitions, queries on the free axis;
#                            lhsT = zero-padded kz block, rhs = qT, fp16)
#   P    = exp(ST)          (ACT engine; host pre-zeroed masked tokens in x,
#                            so masked keys give exp(0)=1 against v=0 rows
#                            and a masked ones-column — they drop out of both
#                            the numerator and the denominator exactly)
#   O^T  = [V | m]^T @ P    (bf16; mask column yields denominators in row 64)
#   r    = 0.5*mask_q * approx_recip(denom)        (DVE, no DMA round trips)
#   bc   = ones64^T @ r     (rank-1 PE broadcast of r over 64 partitions)
#   out += O^T * bc         (DVE, bf16 accumulator)
# The exp on ACT (128 tiles of [128,1024] @ ~1.02us = 131us) is the pipeline
# floor. Schedule: a few junk warm-up matmuls release the PE clock gate while
# the (fp16, host-converted) inputs stream in; all projections then run
# up-front on the hot PE with their PSUM->SBUF casts split across the idle
# ACT engine (kz) and DVE (q, v) so no single cast chain paces the startup;
# the 16 branches pipeline with a one-tile QK lookahead across branches so
# the exp stream runs gap-free (~0.16us/branch residue).

import sys

sys.path.insert(0, "/opt/trn_rl_repo")

import ml_dtypes
import numpy as np

import concourse.bacc as bacc
import concourse.mybir as mybir
import concourse.tile as tile
from concourse.bass_utils import run_bass_kernel_spmd

F32 = mybir.dt.float32
F32R = mybir.dt.float32r
F16 = mybir.dt.float16
BF16 = mybir.dt.bfloat16
EXP = mybir.ActivationFunctionType.Exp

L = 1024  # sequence length (both sides)
D = 512  # hidden
NB = 4  # batches
HPG = 4  # heads per core (head group)
HD = 64  # head size
OG = HPG * HD  # output channels per core = 256
KT = L // 128  # 8 key tiles
DT = D // 128  # 4 contraction tiles for projections
INF = 10000.0

_NC = None  # cached compiled program
TRACE = False  # set by test harness to capture an NTFF profile
LAST_RESULT = None  # full BassKernelResults of the last run (for profiling)
DEBUG_DUMP = False  # dump branch-0 intermediates to DRAM for debugging


def _tt(pool, shape, dtype, tag):
    return pool.tile(shape, dtype, tag=tag, name=tag)


def _install_ntff_hook():
    # antenv.axon_hooks is absent in this image; provide it so
    # run_bass_kernel_spmd(trace=True) can capture NTFF profiles.
    import types, contextlib, ctypes

    if "antenv.axon_hooks" in sys.modules:
        return
    lib = ctypes.CDLL("/opt/axon/libaxon_pjrt.so")
    lib.axon_start_nrt_profile.argtypes = [
        ctypes.POINTER(ctypes.c_int64),
        ctypes.c_size_t,
    ]
    lib.axon_start_nrt_profile.restype = ctypes.c_int64
    lib.axon_stop_nrt_profile.argtypes = [ctypes.c_char_p]
    lib.axon_stop_nrt_profile.restype = ctypes.c_int64

    @contextlib.contextmanager
    def _hook(output_dir, device_ids):
        import jax

        jax.devices()
        if device_ids:
            ids = (ctypes.c_int64 * len(device_ids))(*device_ids)
            rc = lib.axon_start_nrt_profile(ids, len(device_ids))
        else:
            rc = lib.axon_start_nrt_profile(None, 0)
        if rc != 0:
            raise RuntimeError(f"axon_start_nrt_profile rc={rc}")
        try:
            yield
        finally:
            n = lib.axon_stop_nrt_profile(str(output_dir).encode())
            print(f"ntff profile: {n} file(s) in {output_dir}", file=sys.stderr)

    mod = types.ModuleType("antenv.axon_hooks")
    mod.get_axon_ntff_profile_hook = lambda: _hook
    mod.set_axon_ntff_profile_hook = lambda h: None
    sys.modules["antenv.axon_hooks"] = mod


def _build():
    nc = bacc.Bacc("TRN2", target_bir_lowering=False, debug=False, num_devices=8)

    # x and weights arrive as fp16 (host-converted): halves input DMA and
    # makes every projection LDWEIGHTS a cheap 2-byte load, at ~8x finer
    # quantization than bf16 (which overshoots the 2e-2 error budget).
    x_d = {s: nc.declare_dram_parameter(f"x{s}T", [D, L], F16, isOutput=False)
           for s in (1, 2)}
    ws = {}
    for wn in ("wq1", "wk1", "wv1", "wq2", "wk2", "wv2"):
        ws[wn] = nc.declare_dram_parameter(wn, [D, OG], F16, isOutput=False)
    hm_d = {s: nc.declare_dram_parameter(f"hm{s}", [1, L], F32, isOutput=False)
            for s in (1, 2)}
    m4_d = {s: nc.declare_dram_parameter(f"m4{s}", [128, KT * HPG], F32,
                                         isOutput=False) for s in (1, 2)}
    out_d = {s: nc.declare_dram_parameter(f"out{s}T", [OG, L], BF16, isOutput=True)
             for s in (1, 2)}
    dbg = {}
    if DEBUG_DUMP:
        for dn, shape, dt in (
            ("d_st", [128, L], F32), ("d_pt", [128, L], BF16),
            ("d_acc", [HD + 1, L], F32), ("d_rr", [1, L], F32),
            ("d_kz", [128, HPG * KT * 128], F32), ("d_q", [128, 2 * L], F32),
            ("d_ve", [128, HPG * (HD + 1)], BF16),
        ):
            dbg[dn] = nc.declare_dram_parameter(dn, shape, dt, isOutput=True)

    with tile.TileContext(nc) as tc:
        with (
            tc.tile_pool(name="pers", bufs=1) as pers,
            tc.tile_pool(name="pt", bufs=10) as ptp,
            tc.tile_pool(name="sm", bufs=2) as smp,
            tc.tile_pool(name="st", bufs=2, space="PSUM") as stp,
            tc.tile_pool(name="acc", bufs=2, space="PSUM") as accp,
        ):
            # ---- persistent tiles ----
            x_r, w_r, hm_sb, m4_sb, kz, qT = {}, {}, {}, {}, {}, {}
            v_e = {1: [], 2: []}
            for s in (1, 2):
                x_r[s] = _tt(pers, [128, DT, L], F16, f"x{s}")
                hm_sb[s] = _tt(pers, [1, L], F32, f"hm{s}")
                m4_sb[s] = _tt(pers, [128, KT, HPG], F32, f"m4{s}")
                kz[s] = _tt(pers, [128, HPG * KT * 128], F16, f"kz{s}")
                nc.gpsimd.memset(kz[s][:], 0.0)
                qT[s] = _tt(pers, [128, 2, L], F16, f"q{s}T")
            for wn in ws:
                w_r[wn] = _tt(pers, [128, DT, OG], F16, wn)
            outacc = {qs: _tt(pers, [HD, HPG, L], BF16, f"out{qs}")
                      for qs in (1, 2)}

            def emit_dmas(s):
                # split the load across both HWDGE queues (SP + ACT): the
                # scalar engine is idle this early, and two queues halve the
                # serial descriptor-generation wait before projections start.
                e1, e2 = nc.sync, nc.sync
                # the first projection piece (k, ot0) needs only wk's first
                # 128 output columns plus x: land those first so the PE can
                # start right behind the clock-gate warm-up.
                wk, wq = f"wk{s}", f"wq{s}"
                e1.dma_start(w_r[wk][:, :, 0:128],
                             ws[wk][:, 0:128].rearrange("(dk p) c -> p dk c", p=128))
                e1.dma_start(x_r[s][:, 0:2, :],
                             x_d[s][0:256, :].rearrange("(dk p) c -> p dk c", p=128))
                e2.dma_start(x_r[s][:, 2:4, :],
                             x_d[s][256:512, :].rearrange("(dk p) c -> p dk c", p=128))
                e1.dma_start(w_r[wq][:, :, 0:128],
                             ws[wq][:, 0:128].rearrange("(dk p) c -> p dk c", p=128))
                e1.dma_start(w_r[wk][:, :, 128:256],
                             ws[wk][:, 128:256].rearrange("(dk p) c -> p dk c", p=128))
                e1.dma_start(w_r[wq][:, :, 128:256],
                             ws[wq][:, 128:256].rearrange("(dk p) c -> p dk c", p=128))
                e1.dma_start(w_r[f"wv{s}"][:],
                             ws[f"wv{s}"][:].rearrange("(dk p) c -> p dk c", p=128))
                e2.dma_start(hm_sb[s][:], hm_d[s][:])
                e2.dma_start(m4_sb[s][:].rearrange("p a b -> p (a b)"),
                             m4_d[s][:])

            # ---- projections ----
            # qT per side: [128, 2, L] (tile ht holds heads 2ht, 2ht+1).
            # kz per side: [128, HPG*KT*128] zero-padded per (head, kt) block
            # so QK's moving qT streams all 128 partitions at full rate.
            def proj_qk_piece(s, name, ot):
                w = w_r[(f"wq{s}" if name == "q" else f"wk{s}")]
                ps = _tt(stp, [128, L], F32, "st")
                for dk in range(DT):
                    for nh in range(2):
                        nc.tensor.matmul(
                            ps[:, nh * 512:(nh + 1) * 512],
                            w[:, dk, ot * 128:(ot + 1) * 128],
                            x_r[s][:, dk, nh * 512:(nh + 1) * 512],
                            start=(dk == 0),
                            stop=(dk == DT - 1),
                        )
                # casts run on the otherwise-idle ACT engine so the psum
                # slot rotation (and with it the PE) is never DVE-gated.
                if name == "q":
                    nc.vector.tensor_copy(qT[s][:, ot, :], ps[:])
                else:
                    # head 2ot -> partitions 0:64; head 2ot+1 -> 64:128
                    for hh in range(2):
                        h = 2 * ot + hh
                        po = hh * 64
                        nc.scalar.copy(
                            kz[s][po:po + 64, h * KT * 128:(h + 1) * KT * 128],
                            ps[po:po + 64, :],
                        )

            def proj_qk(s):
                for ot in range(2):
                    proj_qk_piece(s, "k", ot)
                    proj_qk_piece(s, "q", ot)

            # v in natural layout with mask column: [128, HPG, 65] per key tile
            def proj_v(s, lts=range(KT), cast=None):
                w = w_r[f"wv{s}"]
                for lt in lts:
                    ps = _tt(stp, [128, OG], F32, "st")
                    for dk in range(DT):
                        nc.tensor.matmul(
                            ps[:],
                            x_r[s][:, dk, lt * 128:(lt + 1) * 128],
                            w[:, dk, :],
                            start=(dk == 0),
                            stop=(dk == DT - 1),
                        )
                    t = _tt(pers, [128, HPG, HD + 1], BF16, f"v{s}_{lt}")
                    (cast or nc.vector.tensor_copy)(
                        t[:, :, 0:HD], ps[:].rearrange("p (h d) -> p h d", h=HPG)
                    )
                    nc.vector.tensor_copy(t[:, :, HD:HD + 1],
                                          m4_sb[s][:, lt, :, None])
                    assert len(v_e[s]) == lt
                    v_e[s].append(t)

            def dump(dn, src, bounce=False):
                if dn not in dbg:
                    return
                if bounce:  # PSUM source: copy to SBUF first
                    t = _tt(pers, dbg[dn].shape, F32, dn)
                    nc.vector.tensor_copy(t[:], src)
                    src = t[:]
                nc.sync.dma_start(dbg[dn][:], src)

            # ---- attention ----
            # (ks, qs, h) order: the first four branches need only side-1
            # projections, so attention starts while side 2 is still loading.
            branches = [(ks, qs, h) for ks in (1, 2) for qs in (1, 2)
                        for h in range(HPG)]

            def emit_qk(ks, qs, h, kt):
                st = _tt(stp, [128, L], F32, "st")
                blk = (h * KT + kt) * 128
                for nh in range(2):
                    nc.tensor.matmul(
                        st[:, nh * 512:(nh + 1) * 512],
                        kz[ks][:, blk:blk + 128],
                        qT[qs][:, h // 2, nh * 512:(nh + 1) * 512],
                        start=True,
                        stop=True,
                    )
                return st

            def emit_combine(p):
                # deferred one iteration: by now the gpsimd broadcast of r is
                # long done, so these DVE ops never block the pipeline.
                ks, qs, h, acc, rbc = p
                oslice = outacc[qs][:, h, :]
                if ks == 1:
                    nc.vector.tensor_mul(oslice, acc[0:HD, :], rbc[:])
                else:
                    tmp = _tt(smp, [64, L], BF16, "tmp")
                    nc.vector.tensor_mul(tmp[:], acc[0:HD, :], rbc[:])
                    nc.vector.tensor_add(oslice, oslice, tmp[:])
                    nc.sync.dma_start(
                        out_d[qs][h * HD:(h + 1) * HD, :], oslice)

            junk = _tt(pers, [128, 512], F16, "junk")
            nc.vector.memset(junk[:], 1.0)
            emit_dmas(1)
            emit_dmas(2)
            # warm-up matmuls on junk data while the input DMAs land: the PE
            # clock gate releases only after sustained activity, so this buys
            # full-speed projections instead of a cold 0.65 GHz start.
            for _ in range(10):
                wps = _tt(stp, [128, 512], F32, "st")
                nc.tensor.matmul(wps[:], junk[:, 0:128], junk[:], start=True,
                                 stop=True)
            # side-1 projections run up front on the warmed-up PE; k/q ot1
            # feed branches 2-3 and are deferred into the branch stream.
            proj_qk_piece(1, "k", 0)
            proj_qk_piece(1, "q", 0)
            proj_qk_piece(1, "k", 1)
            proj_qk_piece(1, "q", 1)
            proj_v(1)
            proj_qk_piece(2, "k", 0)
            proj_qk_piece(2, "q", 0)
            proj_qk_piece(2, "k", 1)
            proj_qk_piece(2, "q", 1)
            proj_v(2)
            if dbg:
                dump("d_kz", kz[1][:], bounce=True)
                dump("d_q", qT[1][:].rearrange("p a b -> p (a b)"), bounce=True)
                dump("d_ve", v_e[1][0][:].rearrange("p a b -> p (a b)"))

            pend = None
            look_st = None
            for bi, (ks, qs, h) in enumerate(branches):
                # QK for all 8 key tiles first: the PE free-runs one tile
                # ahead of ACT (throttled by the two st PSUM slots).
                sts = [look_st] if look_st is not None else []
                for kt in range(len(sts), KT):
                    sts.append(emit_qk(ks, qs, h, kt))
                pts = []
                for kt in range(KT):
                    pt = _tt(ptp, [128, L], BF16, "pt")
                    if bi == 0 and kt == 0:
                        dump("d_st", sts[kt][:], bounce=True)
                    nc.scalar.activation(pt[:], sts[kt][:], EXP)
                    if bi == 0 and kt == 0:
                        dump("d_pt", pt[:])
                    pts.append(pt)
                if pend is not None:
                    emit_combine(pend)
                acc = _tt(accp, [HD + 1, L], F32, "acc")
                for kt in range(KT):
                    if kt == KT - 1 and bi + 1 < len(branches):
                        # software-pipeline: next branch's first QK goes ahead
                        # of this branch's last PV so ACT rolls over gap-free.
                        look_st = emit_qk(*branches[bi + 1][:3], 0)
                    for nh in range(2):
                        nc.tensor.matmul(
                            acc[:, nh * 512:(nh + 1) * 512],
                            v_e[ks][kt][:, h, :],
                            pts[kt][:, nh * 512:(nh + 1) * 512],
                            start=(kt == 0),
                            stop=(kt == KT - 1),
                        )
                if bi == 0:
                    dump("d_acc", acc[:], bounce=True)
                if bi == len(branches) - 1:
                    # the last branch's normalize chain is fully exposed in
                    # the kernel tail: split it into column halves so the
                    # gpsimd broadcast overlaps the DVE ops and each output
                    # half ships as soon as it is combined.
                    oslice = outacc[qs][:, h, :]
                    rbc_h = []
                    for nh in range(2):
                        sl = slice(nh * 512, (nh + 1) * 512)
                        sh = _tt(smp, [1, 512], F32, "s_h")
                        nc.vector.tensor_copy(sh[:], acc[HD:HD + 1, sl])
                        rih = _tt(smp, [1, 512], F32, "ri_h")
                        nc.vector.reciprocal_approx_fast(rih[:], sh[:])
                        rrh = _tt(smp, [1, 512], F32, "rr_h")
                        nc.vector.tensor_mul(rrh[:], rih[:], hm_sb[qs][:, sl])
                        rb = _tt(smp, [64, 512], F32, "rbc_h")
                        nc.gpsimd.partition_broadcast(rb[:], rrh[:])
                        rbc_h.append(rb)
                    for nh in range(2):
                        sl = slice(nh * 512, (nh + 1) * 512)
                        tmp = _tt(smp, [64, 512], BF16, "tmp_h")
                        nc.vector.tensor_mul(tmp[:], acc[0:HD, sl], rbc_h[nh][:])
                        nc.vector.tensor_add(oslice[:, sl], oslice[:, sl], tmp[:])
                        nc.sync.dma_start(out_d[qs][h * HD:(h + 1) * HD, sl],
                                          oslice[:, sl])
                    pend = None
                    continue
                # normalization scalars: r = 0.5 * mask_q / denom, in [1, L],
                # broadcast to 64 partitions on the (otherwise idle) gpsimd.
                s_sb = _tt(smp, [1, L], F32, "s_sb")
                nc.vector.tensor_copy(s_sb[:], acc[HD:HD + 1, :])
                rinv = _tt(smp, [1, L], F32, "rinv")
                nc.vector.reciprocal_approx_fast(rinv[:], s_sb[:])
                rr = _tt(smp, [1, L], F32, "rr")
                nc.vector.tensor_mul(rr[:], rinv[:], hm_sb[qs][:])
                rbc = _tt(smp, [64, L], F32, "rbc")
                nc.gpsimd.partition_broadcast(rbc[:], rr[:])
                if bi == 0:
                    dump("d_rr", rr[:])
                pend = (ks, qs, h, acc, rbc)
            if pend is not None:
                emit_combine(pend)

    nc.compile()
    return nc


def kernel(**inputs):
    global _NC
    if _NC is None:
        _NC = _build()

    mask1 = np.asarray(inputs["mask1"], dtype=np.float32)
    mask2 = np.asarray(inputs["mask2"], dtype=np.float32)
    # pre-zero masked tokens: masked keys then contribute exp(0)*0 = 0 to
    # both the attention numerator and (via the v mask column) denominator.
    x1 = np.asarray(inputs["input1"], dtype=np.float32) * mask1[:, :, None]
    x2 = np.asarray(inputs["input2"], dtype=np.float32) * mask2[:, :, None]
    W = {k: np.asarray(inputs[k], dtype=np.float32) for k in
         ("Wq1", "Wk1", "Wv1", "Wq2", "Wk2", "Wv2")}

    in_maps = []
    for core in range(8):
        b, hg = core // 2, core % 2
        og = slice(hg * OG, (hg + 1) * OG)
        m = {
            "x1T": np.ascontiguousarray(x1[b].T.astype(np.float16)),
            "x2T": np.ascontiguousarray(x2[b].T.astype(np.float16)),
            "hm1": np.ascontiguousarray((0.5 * mask1[b]).reshape(1, L)),
            "hm2": np.ascontiguousarray((0.5 * mask2[b]).reshape(1, L)),
            "m41": np.ascontiguousarray(
                np.repeat(mask1[b].reshape(KT, 128).T[:, :, None], HPG, axis=2)
                .reshape(128, KT * HPG)),
            "m42": np.ascontiguousarray(
                np.repeat(mask2[b].reshape(KT, 128).T[:, :, None], HPG, axis=2)
                .reshape(128, KT * HPG)),
        }
        for wn in ("q1", "k1", "v1", "q2", "k2", "v2"):
            m["w" + wn] = np.ascontiguousarray(
                W["W" + wn[0] + wn[1]].T[:, og].astype(np.float16))
        in_maps.append(m)

    global LAST_RESULT
    if TRACE:
        _install_ntff_hook()
    res = run_bass_kernel_spmd(_NC, in_maps, list(range(8)), trace=TRACE)
    LAST_RESULT = res

    output1 = np.empty((NB, L, D), dtype=np.float32)
    output2 = np.empty((NB, L, D), dtype=np.float32)
    for core in range(8):
        b, hg = core // 2, core % 2
        og = slice(hg * OG, (hg + 1) * OG)
        output1[b, :, og] = np.asarray(res.results[core]["out1T"],
                                       dtype=np.float32).T
        output2[b, :, og] = np.asarray(res.results[core]["out2T"],
                                       dtype=np.float32).T
    return (output1, output2)

